# revision 16
# baseline (speedup 1.0000x reference)
"""DigitCapsule (dynamic routing) Trainium2 Bass kernel, v2.

Problem: x (128,1152,8) f32, W (1,1152,10,16,8) f32 ->
  u_hat[b,r,o,do] = sum_di W[r,o,do,di] x[b,r,di]
  3 routing iterations (softmax over routes r, squash), output v (128,10,16,1).

Sharding: data-parallel over batch, 16 samples per core, W replicated.

Per-core layout (partition p = 16*j + b, j = r mod 8, b = batch-in-core):
  u[p, cc, do, o] = u_hat[b, 8*cc+j, o, do]   (fp16, 144 x 16 x 10 free)

v2 changes vs v1:
  - block-diagonal stationary X is precomputed on host and DMA'd (both X and
    W tiles use 128 partitions, chunked + double-buffered) -> no 19us memset.
  - s0 accumulation chain interleaved with production groups.
  - agreement do-reduction moved from a VectorE adder tree to PE: 16
    accumulating matmuls per cc-batch with an identity stationary, summing
    u*v over do directly in PSUM.
  - squash simplified to v = s*|s|/(1+s^2) (exact up to the reference's
    1e-9 epsilon smoothing), all on VectorE with fast reciprocal.
  - PSUM evacuation split across Scalar/Vector/GpSimd; premult batches mostly
    on VectorE (fp16 2x mode) with one batch per pass on GpSimd.
"""

import numpy as np

import concourse.bacc as bacc
import concourse.bass as bass
import concourse.tile as tile
from concourse import mybir
from concourse.bass_utils import run_bass_kernel_spmd

B, R, O, DO, DI = 128, 1152, 10, 16, 8
NCORES = 8
BC = B // NCORES          # 16 samples per core
J = 8                     # routes per matmul group
CC = R // J               # 144 cc groups
CCG = CC // 2             # 72 stationary-pair groups
OD = O * DO               # 160
F16 = mybir.dt.float16
F32 = mybir.dt.float32

PROD_GROUP = 2            # cc per production psum tile (1 bank per cc)
NPG = CC // PROD_GROUP    # 24 production groups
CHUNK = 12                # ccg per DMA chunk (= 4 production groups)
NCH = CCG // CHUNK        # 6 chunks
TB = 24                   # cc per premult batch
NTB = CC // TB            # 6 batches


def _squash(nc, pool, s_src, v_out, scale):
    """v_out = squash(s_src * scale) = z*|z|/(1+z^2), z = s*scale.

    Matches the reference's elementwise squash up to its 1e-9 epsilon.
    s_src may be PSUM f32; v_out any dtype.
    """
    P = v_out.shape[0]
    if scale != 1.0:
        z = pool.tile([P, DO, O], F32, tag="sq_z")
        nc.vector.tensor_scalar_mul(z[:], s_src[:], scale)
        s = z
    else:
        s = s_src
    a = pool.tile([P, DO, O], F32, tag="sq_a")
    m = pool.tile([P, DO, O], F32, tag="sq_m")
    r = pool.tile([P, DO, O], F32, tag="sq_r")
    n = pool.tile([P, DO, O], F32, tag="sq_n")
    ng = pool.tile([P, DO, O], F32, tag="sq_ng")
    nc.vector.tensor_scalar_mul(ng[:], s[:], -1.0)
    nc.vector.tensor_max(a[:], s[:], ng[:])         # |s| (s may be PSUM, ng is SBUF)
    nc.vector.tensor_mul(n[:], s[:], a[:])          # s*|s|
    nc.vector.tensor_mul(m[:], a[:], a[:])          # s^2 (= |s|^2, avoids 2nd PSUM read)
    nc.vector.tensor_scalar_add(m[:], m[:], 1.0)    # 1+s^2
    nc.vector.reciprocal(r[:], m[:])
    nc.vector.tensor_mul(v_out[:], n[:], r[:])


def build_nc():
    nc = bacc.Bacc("TRN2", debug=False)
    wt_d = nc.dram_tensor("wt", [128, CCG, DO, O], F16, kind="ExternalInput")
    xd_d = nc.dram_tensor("xd", [128, CCG, 128], F16, kind="ExternalInput")
    d16_d = nc.dram_tensor("d16", [128, 128], F16, kind="ExternalInput")
    d32_d = nc.dram_tensor("d32", [128, 128], F32, kind="ExternalInput")
    dout_d = nc.dram_tensor("dout", [128, BC], F16, kind="ExternalInput")
    id16_d = nc.dram_tensor("id16", [128, 128], F16, kind="ExternalInput")
    out_d = nc.dram_tensor("out", [BC, O, DO], F32, kind="ExternalOutput")

    with tile.TileContext(nc) as tc:
        with (
            tc.tile_pool(name="const", bufs=1) as const,
            tc.tile_pool(name="wtp", bufs=3) as wtp,
            tc.tile_pool(name="xdp", bufs=3) as xdp,
            tc.tile_pool(name="main", bufs=1) as main,
            tc.tile_pool(name="sq", bufs=2) as sq,
            tc.tile_pool(name="tp", bufs=4) as tp,
            tc.tile_pool(name="pp", bufs=2, space=bass.MemorySpace.PSUM) as pp,
            tc.tile_pool(name="pa", bufs=2, space=bass.MemorySpace.PSUM) as pa,
            tc.tile_pool(name="pss", bufs=1, space=bass.MemorySpace.PSUM) as pss,
        ):
            zero = const.tile([128, 1], F32)
            nc.vector.memset(zero[:], 0.0)
            d16 = const.tile([128, 128], F16)
            d32 = const.tile([128, 128], F32)
            dout = const.tile([128, BC], F16)
            id16 = const.tile([128, 128], F16)
            nc.sync.dma_start(d16[:], d16_d[:])
            nc.sync.dma_start(d32[:], d32_d[:])
            nc.sync.dma_start(dout[:], dout_d[:])
            nc.sync.dma_start(id16[:], id16_d[:])

            u = main.tile([128, CC, DO, O], F16)
            s0_ps = pss.tile([128, DO, O], F32, tag="s")

            # ---- production (+ interleaved s0 chain), chunked DMA ----
            wt_ch = {}
            xd_ch = {}
            for ch in range(NCH):
                sl = slice(ch * CHUNK, (ch + 1) * CHUNK)
                wt_ch[ch] = wtp.tile([128, CHUNK, DO, O], F16, tag="wt",
                                     name=f"wt{ch}")
                xd_ch[ch] = xdp.tile([128, CHUNK, 128], F16, tag="xd",
                                     name=f"xd{ch}")
                nc.sync.dma_start(wt_ch[ch][:], wt_d[:, sl, :, :])
                nc.sync.dma_start(xd_ch[ch][:], xd_d[:, sl, :])

            evac_eng = [nc.scalar, nc.vector]
            for g in range(NPG):
                ch = (g * PROD_GROUP // 2) // CHUNK
                wt = wt_ch[ch]
                xd = xd_ch[ch]
                ps = pp.tile([128, PROD_GROUP, 512], F32, tag="pp")
                for i in range(PROD_GROUP):
                    cc = g * PROD_GROUP + i
                    ccg, ccp = cc // 2, cc % 2
                    lccg = ccg - ch * CHUNK
                    nc.tensor.matmul(
                        ps[:, i, 0:OD],
                        xd[64 * ccp : 64 * ccp + 64, lccg, :],
                        wt[64 * ccp : 64 * ccp + 64, lccg, :, :],
                        start=True, stop=True,
                    )
                sl = slice(g * PROD_GROUP, (g + 1) * PROD_GROUP)
                src = ps[:, :, 0:OD].rearrange("p c (do o) -> p c do o", do=DO)
                eng = evac_eng[g % 2]
                if eng is nc.scalar:
                    nc.scalar.copy(u[:, sl, :, :], src)
                else:
                    eng.tensor_copy(u[:, sl, :, :], src)

            # s0 = sum_r u (uniform c), contiguous accumulation chain
            for cc in range(CC):
                nc.tensor.matmul(
                    s0_ps[:], d16[:], u[:, cc, :, :],
                    start=(cc == 0), stop=(cc == CC - 1),
                )

            v = main.tile([128, DO, O], F16)
            _squash(nc, sq, s0_ps, v, 1.0 / R)

            e = main.tile([128, CC, O], F32)
            ea = main.tile([128, CC, O], F32)
            e_r = main.tile([128, O], F32)
            inv = main.tile([128, O], F32)
            c16 = main.tile([128, CC, O], F16)

            for it in (1, 2):
                final = it == 2
                # ---- agreement: a = sum_do u*v on PE; e (*)= exp(a) ----
                for g in range(NTB):
                    sl = slice(g * TB, (g + 1) * TB)
                    t = tp.tile([128, TB, DO, O], F16, tag="t")
                    v_b = v[:].unsqueeze(1).broadcast_to((128, TB, DO, O))
                    nc.vector.tensor_mul(t[:], u[:, sl, :, :], v_b)
                    a_ps = pa.tile([128, TB, O], F32, tag="a")
                    for do in range(DO):
                        nc.tensor.matmul(
                            a_ps[:], id16[:], t[:, :, do, :],
                            start=(do == 0), stop=(do == DO - 1),
                        )
                    dst = e if it == 1 else ea
                    nc.scalar.activation(dst[:, sl, :], a_ps[:],
                                         mybir.ActivationFunctionType.Exp,
                                         bias=zero[:])
                    if it == 2:
                        nc.vector.tensor_mul(e[:, sl, :], e[:, sl, :], ea[:, sl, :])

                # ---- softmax over routes ----
                e_perm = e[:].transpose((0, 2, 1))
                nc.vector.reduce_sum(e_r[:], e_perm, axis=mybir.AxisListType.X)
                den = pss.tile([128, DO, O], F32, tag="s", name="den")
                nc.tensor.matmul(den[:, 0, :], d32[:], e_r[:], start=True, stop=True)
                nc.vector.reciprocal(inv[:], den[:, 0, :])

                # ---- s = sum_r c*u (premult on DVE/Pool, sum on PE) ----
                sp_p = BC if final else 128
                lhs = dout if final else d16
                s_ps2 = pss.tile([sp_p, DO, O], F32, tag="s")
                for g in range(NTB):
                    sl = slice(g * TB, (g + 1) * TB)
                    inv_b = inv[:].unsqueeze(1).broadcast_to((128, TB, O))
                    nc.vector.tensor_mul(c16[:, sl, :], e[:, sl, :], inv_b)
                    t = tp.tile([128, TB, DO, O], F16, tag="t")
                    c_b = c16[:, sl, :].unsqueeze(2).broadcast_to(
                        (128, TB, DO, O))
                    nc.vector.tensor_mul(t[:], u[:, sl, :, :], c_b)
                    for i in range(TB):
                        cc = g * TB + i
                        nc.tensor.matmul(
                            s_ps2[:], lhs[:, :sp_p], t[:, i, :, :],
                            start=(cc == 0), stop=(cc == CC - 1),
                        )
                if not final:
                    _squash(nc, sq, s_ps2, v, 1.0)
                else:
                    v2 = main.tile([BC, DO, O], F32)
                    _squash(nc, sq, s_ps2, v2, 1.0)
                    v2p = main.tile([BC, O, DO], F32)
                    nc.vector.tensor_copy(v2p[:], v2[:].transpose((0, 2, 1)))
                    nc.sync.dma_start(out_d[:], v2p[:])

    nc.compile()
    return nc


_CACHE = {}


def _get_nc():
    if "nc" not in _CACHE:
        _CACHE["nc"] = build_nc()
    return _CACHE["nc"]


def _prep_const():
    if "const" not in _CACHE:
        p = np.arange(128)
        d16 = (p[:, None] % 16 == p[None, :] % 16).astype(np.float16)
        d32 = d16.astype(np.float32)
        dout = (p[:, None] % 16 == np.arange(BC)[None, :]).astype(np.float16)
        id16 = np.eye(128, dtype=np.float16)
        _CACHE["const"] = (d16, d32, dout, id16)
    return _CACHE["const"]


def kernel(x: np.ndarray, W: np.ndarray) -> np.ndarray:
    x = np.asarray(x, dtype=np.float32)
    W = np.asarray(W, dtype=np.float32)
    nc = _get_nc()
    d16, d32, dout, id16 = _prep_const()
    W5 = W.reshape(R, O, DO, DI)
    # wt[64*ccp + 8j + di, ccg, do, o] = W[8*(2*ccg+ccp)+j, o, do, di]
    wt = np.ascontiguousarray(
        W5.reshape(CCG, 2, J, O, DO, DI).transpose(1, 2, 5, 0, 4, 3)
    ).reshape(128, CCG, DO, O).astype(np.float16)
    in_maps = []
    for q in range(NCORES):
        xq = x[BC * q : BC * (q + 1)]                # [16, 1152, 8]
        xr = xq.reshape(BC, CCG, 2, J, DI)           # [b, ccg, ccp, j, di]
        xd = np.zeros((2, J, DI, CCG, J, BC), np.float16)
        for j in range(J):
            xd[:, j, :, :, j, :] = xr[:, :, :, j, :].transpose(2, 3, 1, 0)
        xd = xd.reshape(128, CCG, 128)
        in_maps.append({"wt": wt, "xd": xd, "d16": d16, "d32": d32,
                        "dout": dout, "id16": id16})
    res = run_bass_kernel_spmd(nc, in_maps, core_ids=list(range(NCORES)))
    out = np.concatenate([res.results[q]["out"] for q in range(NCORES)], axis=0)
    return out.reshape(B, O, DO, 1).astype(np.float32)


# revision 17
# speedup vs baseline: 1.0927x; 1.0927x over previous
"""DigitCapsule (dynamic routing) Trainium2 Bass kernel, v3.

Problem: x (128,1152,8) f32, W (1,1152,10,16,8) f32 ->
  u_hat[b,r,o,do] = sum_di W[r,o,do,di] x[b,r,di]
  3 routing iterations (softmax over routes r, squash), output v (128,10,16,1).

Sharding: data-parallel over batch, 16 samples per core, W replicated.

Per-core layout (partition p = 16*j + b, r = 16*ccg + 8*ccp + j,
cc = 2*ccg + ccp):
  u[p, cc, do, o] = u_hat[b, r, o, do]   (fp16, 144 x 16 x 10 free)

Design notes:
  - block-diagonal stationary X precomputed on host, DMA'd in chunks with the
    W tiles (128 partitions each, double-buffered) -> no on-chip memset.
  - agreement do-reduction on PE: 16 accumulating identity-stationary matmuls
    per cc-batch summing u*v over do directly in PSUM.
  - routing logits kept as e = exp(b) (f32): per-batch Exp reads agreement
    PSUM on Act; iteration 2 multiplies exp(a2) in on DVE. No b_ij tensor,
    no logit evacuation.
  - squash simplified to v = s*|s|/(1+s^2) (exact up to the reference's
    1e-9 epsilon smoothing), all on VectorE.
  - PSUM pools are phase-scoped: production uses a 3-deep 2-bank pipeline,
    the routing iterations use a 2-buffer agreement pool + 1-bank s/den slot.
"""

import numpy as np

import concourse.bacc as bacc
import concourse.bass as bass
import concourse.tile as tile
from concourse import mybir
from concourse.bass_utils import run_bass_kernel_spmd

B, R, O, DO, DI = 128, 1152, 10, 16, 8
NCORES = 8
BC = B // NCORES          # 16 samples per core
J = 8                     # routes per matmul group
CC = R // J               # 144 cc groups
CCG = CC // 2             # 72 stationary-pair groups
OD = O * DO               # 160
F16 = mybir.dt.float16
F32 = mybir.dt.float32

PROD_GROUP = 2            # cc per production psum tile (1 bank per cc)
NPG = CC // PROD_GROUP    # production groups
CHUNK = 12                # ccg per DMA chunk
NCH = CCG // CHUNK        # 6 chunks
TB = 24                   # cc per premult batch
NTB = CC // TB            # 6 batches


def _squash(nc, pool, s_src, v_out, scale):
    """v_out = squash(s_src * scale) = z*|z|/(1+z^2), z = s*scale.

    Matches the reference's elementwise squash up to its 1e-9 epsilon.
    s_src may be PSUM f32; v_out any dtype.
    """
    P = v_out.shape[0]
    if scale != 1.0:
        z = pool.tile([P, DO, O], F32, tag="sq_z")
        nc.vector.tensor_scalar_mul(z[:], s_src[:], scale)
        s = z
    else:
        s = s_src
    a = pool.tile([P, DO, O], F32, tag="sq_a")
    m = pool.tile([P, DO, O], F32, tag="sq_m")
    r = pool.tile([P, DO, O], F32, tag="sq_r")
    n = pool.tile([P, DO, O], F32, tag="sq_n")
    ng = pool.tile([P, DO, O], F32, tag="sq_ng")
    nc.vector.tensor_scalar_mul(ng[:], s[:], -1.0)
    nc.vector.tensor_max(a[:], s[:], ng[:])         # |s| (s may be PSUM, ng is SBUF)
    nc.vector.tensor_mul(n[:], s[:], a[:])          # s*|s|
    nc.vector.tensor_mul(m[:], a[:], a[:])          # s^2 (avoids a 2nd PSUM read)
    nc.vector.tensor_scalar_add(m[:], m[:], 1.0)    # 1+s^2
    nc.vector.reciprocal(r[:], m[:])
    nc.vector.tensor_mul(v_out[:], n[:], r[:])


def build_nc():
    nc = bacc.Bacc("TRN2", debug=False)
    wt_d = nc.dram_tensor("wt", [128, CCG, DO, O], F16, kind="ExternalInput")
    xd_d = nc.dram_tensor("xd", [128, CCG, 128], F16, kind="ExternalInput")
    d16_d = nc.dram_tensor("d16", [128, 128], F16, kind="ExternalInput")
    d32_d = nc.dram_tensor("d32", [128, 128], F32, kind="ExternalInput")
    dout_d = nc.dram_tensor("dout", [128, BC], F16, kind="ExternalInput")
    id16_d = nc.dram_tensor("id16", [128, 128], F16, kind="ExternalInput")
    out_d = nc.dram_tensor("out", [BC, O, DO], F32, kind="ExternalOutput")

    with tile.TileContext(nc) as tc:
        with (
            tc.tile_pool(name="const", bufs=1) as const,
            tc.tile_pool(name="wtp", bufs=3) as wtp,
            tc.tile_pool(name="xdp", bufs=3) as xdp,
            tc.tile_pool(name="main", bufs=1) as main,
            tc.tile_pool(name="sq", bufs=2) as sq,
            tc.tile_pool(name="tp", bufs=4) as tp,
            tc.tile_pool(name="pss", bufs=1, space=bass.MemorySpace.PSUM) as pss,
        ):
            zero = const.tile([128, 1], F32)
            nc.vector.memset(zero[:], 0.0)
            d16 = const.tile([128, 128], F16)
            d32 = const.tile([128, 128], F32)
            dout = const.tile([128, BC], F16)
            id16 = const.tile([128, 128], F16)
            nc.sync.dma_start(d16[:], d16_d[:])
            nc.sync.dma_start(d32[:], d32_d[:])
            nc.sync.dma_start(dout[:], dout_d[:])
            nc.sync.dma_start(id16[:], id16_d[:])

            u = main.tile([128, CC, DO, O], F16)
            s0_ps = pss.tile([128, DO, O], F32, tag="s")

            # ---- chunked input DMA ----
            wt_ch = {}
            xd_ch = {}
            for ch in range(NCH):
                sl = slice(ch * CHUNK, (ch + 1) * CHUNK)
                wt_ch[ch] = wtp.tile([128, CHUNK, DO, O], F16, tag="wt",
                                     name=f"wt{ch}")
                xd_ch[ch] = xdp.tile([128, CHUNK, 128], F16, tag="xd",
                                     name=f"xd{ch}")
                nc.sync.dma_start(wt_ch[ch][:], wt_d[:, sl, :, :])
                nc.sync.dma_start(xd_ch[ch][:], xd_d[:, sl, :])

            # ---- production, 3-deep psum pipeline ----
            evac_eng = [nc.scalar, nc.vector]
            with tc.tile_pool(name="pp", bufs=3,
                              space=bass.MemorySpace.PSUM) as pp:
                for g in range(NPG):
                    ch = (g * PROD_GROUP // 2) // CHUNK
                    wt = wt_ch[ch]
                    xd = xd_ch[ch]
                    ps = pp.tile([128, PROD_GROUP, 512], F32, tag="pp")
                    for i in range(PROD_GROUP):
                        cc = g * PROD_GROUP + i
                        ccg, ccp = cc // 2, cc % 2
                        lccg = ccg - ch * CHUNK
                        nc.tensor.matmul(
                            ps[:, i, 0:OD],
                            xd[64 * ccp : 64 * ccp + 64, lccg, :],
                            wt[64 * ccp : 64 * ccp + 64, lccg, :, :],
                            start=True, stop=True,
                        )
                    sl = slice(g * PROD_GROUP, (g + 1) * PROD_GROUP)
                    src = ps[:, :, 0:OD].rearrange("p c (do o) -> p c do o", do=DO)
                    eng = evac_eng[g % 2]
                    if eng is nc.scalar:
                        nc.scalar.copy(u[:, sl, :, :], src)
                    else:
                        eng.tensor_copy(u[:, sl, :, :], src)

                # s0 = sum_r u (uniform c), contiguous accumulation chain
                for cc in range(CC):
                    nc.tensor.matmul(
                        s0_ps[:], d16[:], u[:, cc, :, :],
                        start=(cc == 0), stop=(cc == CC - 1),
                    )

            v = main.tile([128, DO, O], F16)
            _squash(nc, sq, s0_ps, v, 1.0 / R)

            e = main.tile([128, CC, O], F32)
            ea = main.tile([128, CC, O], F32)
            e_r = main.tile([128, O], F32)
            inv = main.tile([128, O], F32)
            c16 = main.tile([128, CC, O], F16)

            with tc.tile_pool(name="pa", bufs=2,
                              space=bass.MemorySpace.PSUM) as pa:
                for it in (1, 2):
                    final = it == 2
                    # ---- agreement: a = sum_do u*v on PE; e (*)= exp(a) ----
                    for g in range(NTB):
                        sl = slice(g * TB, (g + 1) * TB)
                        t = tp.tile([128, TB, DO, O], F16, tag="t")
                        v_b = v[:].unsqueeze(1).broadcast_to((128, TB, DO, O))
                        nc.vector.tensor_mul(t[:], u[:, sl, :, :], v_b)
                        a_ps = pa.tile([128, TB, O], F32, tag="a")
                        for do in range(DO):
                            nc.tensor.matmul(
                                a_ps[:], id16[:], t[:, :, do, :],
                                start=(do == 0), stop=(do == DO - 1),
                            )
                        dst = e if it == 1 else ea
                        nc.scalar.activation(dst[:, sl, :], a_ps[:],
                                             mybir.ActivationFunctionType.Exp,
                                             bias=zero[:])
                        if it == 2:
                            nc.vector.tensor_mul(e[:, sl, :], e[:, sl, :],
                                                 ea[:, sl, :])

                    # ---- softmax over routes ----
                    e_perm = e[:].transpose((0, 2, 1))
                    nc.vector.reduce_sum(e_r[:], e_perm, axis=mybir.AxisListType.X)
                    den = pss.tile([128, DO, O], F32, tag="s", name="den")
                    nc.tensor.matmul(den[:, 0, :], d32[:], e_r[:],
                                     start=True, stop=True)
                    nc.vector.reciprocal(inv[:], den[:, 0, :])

                    # ---- s = sum_r c*u (premult on DVE, sum on PE) ----
                    sp_p = BC if final else 128
                    lhs = dout if final else d16
                    s_ps2 = pss.tile([sp_p, DO, O], F32, tag="s")
                    for g in range(NTB):
                        sl = slice(g * TB, (g + 1) * TB)
                        inv_b = inv[:].unsqueeze(1).broadcast_to((128, TB, O))
                        nc.vector.tensor_mul(c16[:, sl, :], e[:, sl, :], inv_b)
                        t = tp.tile([128, TB, DO, O], F16, tag="t")
                        c_b = c16[:, sl, :].unsqueeze(2).broadcast_to(
                            (128, TB, DO, O))
                        nc.vector.tensor_mul(t[:], u[:, sl, :, :], c_b)
                        for i in range(TB):
                            cc = g * TB + i
                            nc.tensor.matmul(
                                s_ps2[:], lhs[:, :sp_p], t[:, i, :, :],
                                start=(cc == 0), stop=(cc == CC - 1),
                            )
                    if not final:
                        _squash(nc, sq, s_ps2, v, 1.0)
                    else:
                        v2 = main.tile([BC, DO, O], F32)
                        _squash(nc, sq, s_ps2, v2, 1.0)
                        v2p = main.tile([BC, O, DO], F32)
                        nc.vector.tensor_copy(v2p[:], v2[:].transpose((0, 2, 1)))
                        nc.sync.dma_start(out_d[:], v2p[:])

    nc.compile()
    return nc


_CACHE = {}


def _get_nc():
    if "nc" not in _CACHE:
        _CACHE["nc"] = build_nc()
    return _CACHE["nc"]


def _prep_const():
    if "const" not in _CACHE:
        p = np.arange(128)
        d16 = (p[:, None] % 16 == p[None, :] % 16).astype(np.float16)
        d32 = d16.astype(np.float32)
        dout = (p[:, None] % 16 == np.arange(BC)[None, :]).astype(np.float16)
        id16 = np.eye(128, dtype=np.float16)
        _CACHE["const"] = (d16, d32, dout, id16)
    return _CACHE["const"]


def kernel(x: np.ndarray, W: np.ndarray) -> np.ndarray:
    x = np.asarray(x, dtype=np.float32)
    W = np.asarray(W, dtype=np.float32)
    nc = _get_nc()
    d16, d32, dout, id16 = _prep_const()
    W5 = W.reshape(R, O, DO, DI)
    # wt[64*ccp + 8j + di, ccg, do, o] = W[16*ccg + 8*ccp + j, o, do, di]
    wt = np.ascontiguousarray(
        W5.reshape(CCG, 2, J, O, DO, DI).transpose(1, 2, 5, 0, 4, 3)
    ).reshape(128, CCG, DO, O).astype(np.float16)
    in_maps = []
    for q in range(NCORES):
        xq = x[BC * q : BC * (q + 1)]                # [16, 1152, 8]
        xr = xq.reshape(BC, CCG, 2, J, DI)           # [b, ccg, ccp, j, di]
        xd = np.zeros((2, J, DI, CCG, J, BC), np.float16)
        for j in range(J):
            xd[:, j, :, :, j, :] = xr[:, :, :, j, :].transpose(2, 3, 1, 0)
        xd = xd.reshape(128, CCG, 128)
        in_maps.append({"wt": wt, "xd": xd, "d16": d16, "d32": d32,
                        "dout": dout, "id16": id16})
    res = run_bass_kernel_spmd(nc, in_maps, core_ids=list(range(NCORES)))
    out = np.concatenate([res.results[q]["out"] for q in range(NCORES)], axis=0)
    return out.reshape(B, O, DO, 1).astype(np.float32)


# revision 18
# speedup vs baseline: 1.1841x; 1.0836x over previous
"""DigitCapsule (dynamic routing) Trainium2 Bass kernel, v3.

Problem: x (128,1152,8) f32, W (1,1152,10,16,8) f32 ->
  u_hat[b,r,o,do] = sum_di W[r,o,do,di] x[b,r,di]
  3 routing iterations (softmax over routes r, squash), output v (128,10,16,1).

Sharding: data-parallel over batch, 16 samples per core, W replicated.

Per-core layout (partition p = 16*j + b, r = 16*ccg + 8*ccp + j,
cc = 2*ccg + ccp):
  u[p, cc, do, o] = u_hat[b, r, o, do]   (fp16, 144 x 16 x 10 free)

Design notes:
  - block-diagonal stationary X precomputed on host, DMA'd in chunks with the
    W tiles (128 partitions each, double-buffered) -> no on-chip memset.
  - agreement do-reduction on PE: 16 accumulating identity-stationary matmuls
    per cc-batch summing u*v over do directly in PSUM.
  - routing logits kept as e = exp(b) (f32): per-batch Exp reads agreement
    PSUM on Act; iteration 2 multiplies exp(a2) in on DVE. No b_ij tensor,
    no logit evacuation.
  - squash simplified to v = s*|s|/(1+s^2) (exact up to the reference's
    1e-9 epsilon smoothing), all on VectorE.
  - PSUM pools are phase-scoped: production uses a 3-deep 2-bank pipeline,
    the routing iterations use a 2-buffer agreement pool + 1-bank s/den slot.
"""

import numpy as np

import concourse.bacc as bacc
import concourse.bass as bass
import concourse.tile as tile
from concourse import mybir
from concourse.bass_utils import run_bass_kernel_spmd

B, R, O, DO, DI = 128, 1152, 10, 16, 8
NCORES = 8
BC = B // NCORES          # 16 samples per core
J = 8                     # routes per matmul group
CC = R // J               # 144 cc groups
CCG = CC // 2             # 72 stationary-pair groups
OD = O * DO               # 160
F16 = mybir.dt.float16
F32 = mybir.dt.float32

PROD_GROUP = 2            # cc per production psum tile (1 bank per cc)
NPG = CC // PROD_GROUP    # production groups
CHUNK = 12                # ccg per DMA chunk
NCH = CCG // CHUNK        # 6 chunks
TB = 24                   # cc per premult batch (legacy name)
BATCHES = [12, 24, 36, 36, 24, 12]   # tapered premult batch sizes
BOFF = [0, 12, 36, 72, 108, 132]     # prefix offsets
NTB = len(BATCHES)


def _squash(nc, pool, s_src, v_out, scale):
    """v_out = squash(s_src * scale) = z*|z|/(1+z^2), z = s*scale.

    Matches the reference's elementwise squash up to its 1e-9 epsilon.
    s_src may be PSUM f32; v_out any dtype.
    """
    P = v_out.shape[0]
    if scale != 1.0:
        z = pool.tile([P, DO, O], F32, tag="sq_z")
        nc.vector.tensor_scalar_mul(z[:], s_src[:], scale)
        s = z
    else:
        s = s_src
    a = pool.tile([P, DO, O], F32, tag="sq_a")
    m = pool.tile([P, DO, O], F32, tag="sq_m")
    r = pool.tile([P, DO, O], F32, tag="sq_r")
    n = pool.tile([P, DO, O], F32, tag="sq_n")
    ng = pool.tile([P, DO, O], F32, tag="sq_ng")
    nc.vector.tensor_scalar_mul(ng[:], s[:], -1.0)
    nc.vector.tensor_max(a[:], s[:], ng[:])         # |s| (s may be PSUM, ng is SBUF)
    nc.vector.tensor_mul(n[:], s[:], a[:])          # s*|s|
    nc.vector.tensor_mul(m[:], a[:], a[:])          # s^2 (avoids a 2nd PSUM read)
    nc.vector.tensor_scalar_add(m[:], m[:], 1.0)    # 1+s^2
    nc.vector.reciprocal(r[:], m[:])
    nc.vector.tensor_mul(v_out[:], n[:], r[:])


def build_nc():
    nc = bacc.Bacc("TRN2", debug=False)
    wt_d = nc.dram_tensor("wt", [128, CCG, DO, O], F16, kind="ExternalInput")
    xd_d = nc.dram_tensor("xd", [128, CCG, 128], F16, kind="ExternalInput")
    d16_d = nc.dram_tensor("d16", [128, 128], F16, kind="ExternalInput")
    d32_d = nc.dram_tensor("d32", [128, 128], F32, kind="ExternalInput")
    dout_d = nc.dram_tensor("dout", [128, BC], F16, kind="ExternalInput")
    id16_d = nc.dram_tensor("id16", [128, 128], F16, kind="ExternalInput")
    out_d = nc.dram_tensor("out", [BC, O, DO], F32, kind="ExternalOutput")

    with tile.TileContext(nc) as tc:
        with (
            tc.tile_pool(name="const", bufs=1) as const,
            tc.tile_pool(name="wtp", bufs=3) as wtp,
            tc.tile_pool(name="xdp", bufs=3) as xdp,
            tc.tile_pool(name="main", bufs=1) as main,
            tc.tile_pool(name="sq", bufs=2) as sq,
            tc.tile_pool(name="tp", bufs=6) as tp,
            tc.tile_pool(name="pss", bufs=1, space=bass.MemorySpace.PSUM) as pss,
        ):
            zero = const.tile([128, 1], F32)
            nc.vector.memset(zero[:], 0.0)
            d16 = const.tile([128, 128], F16)
            d32 = const.tile([128, 128], F32)
            dout = const.tile([128, BC], F16)
            id16 = const.tile([128, 128], F16)

            u = main.tile([128, CC, DO, O], F16)
            s0_ps = pss.tile([128, DO, O], F32, tag="s")

            # ---- chunked input DMA (chunk 0 first; consts are needed later) ----
            wt_ch = {}
            xd_ch = {}
            for ch in range(NCH):
                sl = slice(ch * CHUNK, (ch + 1) * CHUNK)
                wt_ch[ch] = wtp.tile([128, CHUNK, DO, O], F16, tag="wt",
                                     name=f"wt{ch}")
                xd_ch[ch] = xdp.tile([128, CHUNK, 128], F16, tag="xd",
                                     name=f"xd{ch}")
                nc.sync.dma_start(xd_ch[ch][:], xd_d[:, sl, :])
                nc.sync.dma_start(wt_ch[ch][:], wt_d[:, sl, :, :])
                if ch == 0:
                    nc.sync.dma_start(d16[:], d16_d[:])
                    nc.sync.dma_start(id16[:], id16_d[:])
                    nc.sync.dma_start(d32[:], d32_d[:])
                    nc.sync.dma_start(dout[:], dout_d[:])

            # ---- production, 3-deep psum pipeline ----
            evac_eng = [nc.scalar, nc.vector]
            with tc.tile_pool(name="pp", bufs=3,
                              space=bass.MemorySpace.PSUM) as pp:
                for g in range(NPG):
                    ch = (g * PROD_GROUP // 2) // CHUNK
                    wt = wt_ch[ch]
                    xd = xd_ch[ch]
                    ps = pp.tile([128, PROD_GROUP, 512], F32, tag="pp")
                    for i in range(PROD_GROUP):
                        cc = g * PROD_GROUP + i
                        ccg, ccp = cc // 2, cc % 2
                        lccg = ccg - ch * CHUNK
                        nc.tensor.matmul(
                            ps[:, i, 0:OD],
                            xd[64 * ccp : 64 * ccp + 64, lccg, :],
                            wt[64 * ccp : 64 * ccp + 64, lccg, :, :],
                            start=True, stop=True,
                        )
                    sl = slice(g * PROD_GROUP, (g + 1) * PROD_GROUP)
                    src = ps[:, :, 0:OD].rearrange("p c (do o) -> p c do o", do=DO)
                    eng = evac_eng[g % 2]
                    if eng is nc.scalar:
                        nc.scalar.copy(u[:, sl, :, :], src)
                    else:
                        eng.tensor_copy(u[:, sl, :, :], src)

                # s0 = sum_r u (uniform c), contiguous accumulation chain
                for cc in range(CC):
                    nc.tensor.matmul(
                        s0_ps[:], d16[:], u[:, cc, :, :],
                        start=(cc == 0), stop=(cc == CC - 1),
                    )

            v = main.tile([128, DO, O], F16)
            _squash(nc, sq, s0_ps, v, 1.0 / R)

            e = main.tile([128, CC, O], F32)
            ea = main.tile([128, CC, O], F32)
            e_r = main.tile([128, O], F32)
            e_rp = main.tile([128, NTB, O], F32)
            inv = main.tile([128, O], F32)
            c16 = main.tile([128, CC, O], F16)

            with tc.tile_pool(name="pa", bufs=2,
                              space=bass.MemorySpace.PSUM) as pa:
                for it in (1, 2):
                    final = it == 2
                    # ---- agreement: a = sum_do u*v on PE; e (*)= exp(a) ----
                    for g in range(NTB):
                        tb = BATCHES[g]
                        sl = slice(BOFF[g], BOFF[g] + tb)
                        t = tp.tile([128, tb, DO, O], F16, tag="t",
                                    name=f"t{g}")
                        v_b = v[:].unsqueeze(1).broadcast_to((128, tb, DO, O))
                        nc.vector.tensor_mul(t[:], u[:, sl, :, :], v_b)
                        a_ps = pa.tile([128, tb, O], F32, tag="a",
                                       name=f"aps{g}")
                        for do in range(DO):
                            nc.tensor.matmul(
                                a_ps[:], id16[:], t[:, :, do, :],
                                start=(do == 0), stop=(do == DO - 1),
                            )
                        dst = e if it == 1 else ea
                        nc.scalar.activation(dst[:, sl, :], a_ps[:],
                                             mybir.ActivationFunctionType.Exp,
                                             bias=zero[:])
                        if it == 2:
                            nc.vector.tensor_mul(e[:, sl, :], e[:, sl, :],
                                                 ea[:, sl, :])
                        ep_g = e[:, sl, :].transpose((0, 2, 1))
                        nc.vector.reduce_sum(e_rp[:, g, :], ep_g,
                                             axis=mybir.AxisListType.X)

                    # ---- softmax over routes ----
                    erp_perm = e_rp[:].transpose((0, 2, 1))
                    nc.vector.reduce_sum(e_r[:], erp_perm,
                                         axis=mybir.AxisListType.X)
                    den = pss.tile([128, DO, O], F32, tag="s", name="den")
                    nc.tensor.matmul(den[:, 0, :], d32[:], e_r[:],
                                     start=True, stop=True)
                    nc.vector.reciprocal(inv[:], den[:, 0, :])

                    # ---- s = sum_r c*u (premult on DVE, sum on PE) ----
                    sp_p = BC if final else 128
                    lhs = dout if final else d16
                    s_ps2 = pss.tile([sp_p, DO, O], F32, tag="s")
                    for g in range(NTB):
                        tb = BATCHES[g]
                        sl = slice(BOFF[g], BOFF[g] + tb)
                        inv_b = inv[:].unsqueeze(1).broadcast_to((128, tb, O))
                        nc.vector.tensor_mul(c16[:, sl, :], e[:, sl, :], inv_b)
                        t = tp.tile([128, tb, DO, O], F16, tag="t",
                                    name=f"ts{g}")
                        c_b = c16[:, sl, :].unsqueeze(2).broadcast_to(
                            (128, tb, DO, O))
                        nc.vector.tensor_mul(t[:], u[:, sl, :, :], c_b)
                        for i in range(tb):
                            cc = BOFF[g] + i
                            nc.tensor.matmul(
                                s_ps2[:], lhs[:, :sp_p], t[:, i, :, :],
                                start=(cc == 0), stop=(cc == CC - 1),
                            )
                    if not final:
                        _squash(nc, sq, s_ps2, v, 1.0)
                    else:
                        v2p = main.tile([BC, O, DO], F32)
                        _squash(nc, sq, s_ps2,
                                v2p[:].transpose((0, 2, 1)), 1.0)
                        nc.sync.dma_start(out_d[:], v2p[:])

    nc.compile()
    return nc


_CACHE = {}


def _get_nc():
    if "nc" not in _CACHE:
        _CACHE["nc"] = build_nc()
    return _CACHE["nc"]


def _prep_const():
    if "const" not in _CACHE:
        p = np.arange(128)
        d16 = (p[:, None] % 16 == p[None, :] % 16).astype(np.float16)
        d32 = d16.astype(np.float32)
        dout = (p[:, None] % 16 == np.arange(BC)[None, :]).astype(np.float16)
        id16 = np.eye(128, dtype=np.float16)
        _CACHE["const"] = (d16, d32, dout, id16)
    return _CACHE["const"]


def kernel(x: np.ndarray, W: np.ndarray) -> np.ndarray:
    x = np.asarray(x, dtype=np.float32)
    W = np.asarray(W, dtype=np.float32)
    nc = _get_nc()
    d16, d32, dout, id16 = _prep_const()
    W5 = W.reshape(R, O, DO, DI)
    # wt[64*ccp + 8j + di, ccg, do, o] = W[16*ccg + 8*ccp + j, o, do, di]
    wt = np.ascontiguousarray(
        W5.reshape(CCG, 2, J, O, DO, DI).transpose(1, 2, 5, 0, 4, 3)
    ).reshape(128, CCG, DO, O).astype(np.float16)
    in_maps = []
    for q in range(NCORES):
        xq = x[BC * q : BC * (q + 1)]                # [16, 1152, 8]
        xr = xq.reshape(BC, CCG, 2, J, DI)           # [b, ccg, ccp, j, di]
        xd = np.zeros((2, J, DI, CCG, J, BC), np.float16)
        for j in range(J):
            xd[:, j, :, :, j, :] = xr[:, :, :, j, :].transpose(2, 3, 1, 0)
        xd = xd.reshape(128, CCG, 128)
        in_maps.append({"wt": wt, "xd": xd, "d16": d16, "d32": d32,
                        "dout": dout, "id16": id16})
    res = run_bass_kernel_spmd(nc, in_maps, core_ids=list(range(NCORES)))
    out = np.concatenate([res.results[q]["out"] for q in range(NCORES)], axis=0)
    return out.reshape(B, O, DO, 1).astype(np.float32)
